# revision 39
# baseline (speedup 1.0000x reference)
"""Trainium2 Bass kernel for nn_BaseTransformer (B=16, C=128, L=1024, H=8, dk=dv=32).

Sharding: pure data-parallel over batch — 8 cores x 2 batches each, no collectives.

Per-core algorithm (PE datapath in bf16 — fp32 matmuls stream at 1/4 rate on
this PE; PSUM accumulation and softmax normalization stay fp32):
  - QK projection: chunks of rows [q h0-3 | q h4-7 | k h0-3 | k h4-7], SCALE and
    q-bias folded in host-side (k bias dropped: softmax-invariant; v bias folded
    into the output bias via W_o @ b_v since softmax rows sum to 1).
  - v is projected TRANSPOSED (x^T @ Wv^T) so the PV matmul needs no transposes.
  - logits are computed transposed (S^T[t,s]) so softmax reduction happens via
    matmul against an all-ones stationary (denominator replicated over each
    head's 32 output partitions); exp runs on ScalarE straight out of PSUM.
  - attention matmuls are packed with tile_position (row-packing for K=32 QK,
    col-packing for M=32 PV and denominator) to use more of the PE array.
  - All compute ops keep out/in0/in1 at identical base partitions.
"""

import os
import numpy as np

B, C, L = 16, 128, 1024
DK, DV, H = 32, 32, 8
SCALE = DK ** (-0.5)
NCORES = 8
BLOC = B // NCORES  # batches per core

_CACHE = {}

# bisect stages: proj < qkexp < pv < norm < full
_STAGES = ["proj", "qkexp", "pv", "norm", "full"]


def _stage():
    return os.environ.get("KSTAGE", "full")


def _stage_ge(s):
    return _STAGES.index(_stage()) >= _STAGES.index(s)


def _split_excess_waits(nc, mybir, cap=1):
    """This container's walrus rejects instructions carrying more than one
    sync-wait command ("Too many sync wait commands" in setupSyncWait), while
    Tile freely attaches several. Move all but `cap` waits of every
    instruction onto injected same-engine NoOps placed immediately before it
    (same block order == same engine queue order, so semantics are identical:
    all waits still complete before the instruction issues)."""
    ctr = 0
    for f in nc.m.functions:
        for blk in f.blocks:
            out = []
            changed = False
            for ins in blk.instructions:
                si = ins.sync_info
                waits = list(si.on_wait) if si and si.on_wait else []
                eng = getattr(ins, "engine", None)
                if len(waits) > cap and eng is not None:
                    for w in waits[:-cap]:
                        nop = mybir.InstNoOp(name=f"I-wsplit-{ctr}")
                        ctr += 1
                        nop.engine = eng
                        nop.sync_info = mybir.SyncInfo(on_wait=[w], on_update=[])
                        out.append(nop)
                    ins.sync_info = mybir.SyncInfo(
                        on_wait=waits[-cap:], on_update=list(si.on_update or [])
                    )
                    changed = True
                out.append(ins)
            if changed:
                blk.instructions = out


def _build_nc():
    import concourse.bass as bass
    import concourse.tile as tile
    from concourse import mybir
    from contextlib import ExitStack

    f32 = mybir.dt.float32
    bf16 = mybir.dt.bfloat16
    nc = bass.Bass()

    x_d = nc.dram_tensor("x_sh", [BLOC, C, L], bf16, kind="ExternalInput")
    wqk_d = nc.dram_tensor("wqk", [C, 4, 128], bf16, kind="ExternalInput")
    bqk_d = nc.dram_tensor("bqk", [128, 2], f32, kind="ExternalInput")
    wv_d = nc.dram_tensor("wv", [C, 256], bf16, kind="ExternalInput")
    wo_d = nc.dram_tensor("wo", [128, 3, 128], bf16, kind="ExternalInput")
    bout_d = nc.dram_tensor("bout", [128, 1], f32, kind="ExternalInput")
    out_d = nc.dram_tensor("out_sh", [BLOC, C, L], f32, kind="ExternalOutput")

    i16 = mybir.dt.int16
    Exp = mybir.ActivationFunctionType.Exp
    mult = mybir.AluOpType.mult
    add = mybir.AluOpType.add

    f8 = mybir.dt.float8e4
    DR = (mybir.MatmulPerfMode.DoubleRowSwInterleave
          if int(os.environ.get("KDRSW", "0"))
          else mybir.MatmulPerfMode.DoubleRow)
    dr_tp = bool(int(os.environ.get("KDRTP", "1")))
    use_dr = bool(int(os.environ.get("KDR", "1")))
    # Logits are shifted by -SHIFT before exp so exp fits fp8e4m3's range
    # (max logit ~11.9, e4m3 max 448 = e^6.1); softmax cancels the shift.
    SHIFT = 6.0 if use_dr else 0.0
    # Schraudolph exp on DVE: st_bits = int16(logit * 128*log2e + (127*128+c)),
    # bitcast int16->bf16 ~= exp(logit). Softmax normalization cancels the
    # common-mode approximation error (validated: full-DVE rel-err 2.9e-3).
    SCH_A = float(128.0 * np.log2(np.e))
    SCH_B = float(127.0 * 128.0 - 5.0 - SCH_A * SHIFT)
    # i-chunk PAIRS (p, pp) routed to DVE-exp, spread evenly over the 8-pair
    # cycle per (b,g,j). ACT pairs write exact exp to fp8 and use DoubleRow
    # PV (one matmul per pair); DVE pairs write bf16 Schraudolph st and use
    # two per-chunk matmuls against fp8 vt slices.
    n_dve = int(os.environ.get("KDVE", "3"))  # of 8 pairs per (b,g,j)
    dve_pairs = set()
    acc = 0
    for u in range(8):
        nxt = ((u + 1) * n_dve) // 8
        if nxt > acc:
            dve_pairs.add((u // 2, u % 2))  # (p, pp)
        acc = nxt

    with tile.TileContext(nc) as tc, ExitStack() as ctx:
        consts = ctx.enter_context(tc.tile_pool(name="consts", bufs=1))
        xp = ctx.enter_context(tc.tile_pool(name="xp", bufs=2))
        qkp = ctx.enter_context(tc.tile_pool(name="qkp", bufs=2))
        vtp = ctx.enter_context(tc.tile_pool(name="vtp", bufs=2))
        stp = ctx.enter_context(tc.tile_pool(name="stp", bufs=4))
        zfp = ctx.enter_context(tc.tile_pool(name="zfp", bufs=2))
        rbp = ctx.enter_context(tc.tile_pool(name="rbp", bufs=3))
        outp = ctx.enter_context(tc.tile_pool(name="outp", bufs=2))
        plbufs = int(os.environ.get("KPLBUFS", "3"))
        cmbufs = int(os.environ.get("KCMBUFS", "2"))
        pbig = ctx.enter_context(tc.tile_pool(name="pbig", bufs=plbufs, space="PSUM"))
        pacc = ctx.enter_context(tc.tile_pool(name="pacc", bufs=2, space="PSUM"))

        wqk_sb = consts.tile([C, 4, 128], bf16, name="wqk_sb")
        bqk_sb = consts.tile([128, 2], f32, name="bqk_sb")
        wv_sb = consts.tile([C, 256], bf16, name="wv_sb")
        wo_sb = consts.tile([128, 3, 128], bf16, name="wo_sb")
        bout_sb = consts.tile([128, 1], f32, name="bout_sb")
        ones_sb = consts.tile([128, 32], bf16, name="ones_sb")
        nc.sync.dma_start(out=wqk_sb, in_=wqk_d[:, :, :])
        nc.sync.dma_start(out=bqk_sb, in_=bqk_d[:, :])
        nc.sync.dma_start(out=wv_sb, in_=wv_d[:, :])
        nc.sync.dma_start(out=wo_sb, in_=wo_d[:, :, :])
        nc.sync.dma_start(out=bout_sb, in_=bout_d[:, :])
        nc.vector.memset(ones_sb, 1.0)
        shift_sb = consts.tile([128, 1], f32, name="shift_sb")
        nc.vector.memset(shift_sb, -6.0)

        vt8s = []
        if use_dr:
            # fp8 V^T for DoubleRow PV. Layout [t, p, ko, hp, hh, 128] with
            # i-chunk = 2p+ko, head = 2hp+hh. The 128-col block per head is
            # [v|1|0|0] for hh=0 and [0|0|v|1] for hh=1, so one DR matmul
            # writes the full 128 psum partitions (ISA: col_grp must be 0xf)
            # and the two heads' matmuls accumulate into the same comb tile.
            for bb in range(BLOC):
                v8 = consts.tile([128, 4, 2, 4, 2, 128], f8, name=f"vt8_{bb}")
                nc.gpsimd.memset(v8, 0.0)
                nc.gpsimd.memset(v8[:, :, :, :, 0, 32:64], 1.0)
                nc.gpsimd.memset(v8[:, :, :, :, 1, 96:128], 1.0)
                vt8s.append(v8)

        denmerge = bool(int(os.environ.get("KDENMERGE", "1")))
        assert denmerge or not use_dr, "KDR=1 requires KDENMERGE=1"
        proj_act = bool(int(os.environ.get("KPROJACT", "1")))
        vt_act = bool(int(os.environ.get("KVTACT", "0")))
        wo_act = bool(int(os.environ.get("KWOACT", "1")))
        Ident = mybir.ActivationFunctionType.Identity
        repeat = int(os.environ.get("KREPEAT", "1"))
        for _rep in range(repeat):
          S = {}
          def _proj(b):
            x_sb = xp.tile([C, L], bf16, name="x_sb")
            nc.sync.dma_start(out=x_sb, in_=x_d[b])

            # ---- QK projection: psum -> (q bias-add | k copy) -> SBUF
            qA = qkp.tile([128, L], bf16, name="qA")
            qB = qkp.tile([128, L], bf16, name="qB")
            kA = qkp.tile([128, L], bf16, name="kA")
            kB = qkp.tile([128, L], bf16, name="kB")
            tgts = [qA, qB, kA, kB]
            for cch in (0, 2, 1, 3):
                ps = pbig.tile([128, L], f32, name="pl")
                for jh in range(2):
                    nc.tensor.matmul(
                        out=ps[:, 512 * jh : 512 * jh + 512],
                        lhsT=wqk_sb[:, cch, :],
                        rhs=x_sb[:, 512 * jh : 512 * jh + 512],
                        start=True, stop=True,
                    )
                if cch < 2:
                    if proj_act:
                        nc.scalar.activation(
                            out=tgts[cch], in_=ps, func=Ident,
                            bias=bqk_sb[:, cch : cch + 1],
                        )
                    else:
                        nc.vector.tensor_scalar_add(
                            out=tgts[cch], in0=ps, scalar1=bqk_sb[:, cch : cch + 1]
                        )
                elif proj_act:
                    nc.scalar.copy(out=tgts[cch], in_=ps)
                else:
                    nc.vector.tensor_copy(out=tgts[cch], in_=ps)

            # ---- V^T projection: vt[t, i, h, d] = v_h[d, 128 i + t]
            # With KDENMERGE, each head's stationary is [v (32 cols) | ones
            # (32 cols)] so the PV matmul also produces the softmax
            # denominator (replicated over 32 partitions) in the same pass.
            vw = 64 if denmerge else 32
            vt = vt8s[b] if use_dr else vtp.tile([128, 8, 8, vw], bf16, name="vt")
            for gq in range(2):
                ps = pbig.tile([128, L], f32, name="pl")
                for q in range(4):
                    nc.tensor.matmul(
                        out=ps[:, 256 * q : 256 * q + 256],
                        lhsT=x_sb[:, 128 * (4 * gq + q) : 128 * (4 * gq + q) + 128],
                        rhs=wv_sb,
                        start=True, stop=True,
                    )
                if use_dr:
                    vsrc = ps.rearrange("p (a k hp hh d) -> p a k hp hh d",
                                        a=2, k=2, hp=4, hh=2, d=32)
                    for hh in range(2):
                        vdst = vt[:, 2 * gq : 2 * gq + 2, :, :, hh,
                                  64 * hh : 64 * hh + 32]
                        cp = nc.scalar.copy if vt_act else nc.vector.tensor_copy
                        cp(out=vdst, in_=vsrc[:, :, :, :, hh, :])
                else:
                    vdst = vt[:, 4 * gq : 4 * gq + 4, :, 0:32]
                    vsrc = ps.rearrange("p (a h d) -> p a h d", h=8, d=32)
                    if vt_act:
                        nc.scalar.copy(out=vdst, in_=vsrc)
                    else:
                        nc.vector.tensor_copy(out=vdst, in_=vsrc)
            if denmerge and not use_dr:
                nc.gpsimd.memset(vt[:, :, :, 32:64], 1.0)

            S[b] = dict(x_sb=x_sb, qA=qA, qB=qB, kA=kA, kB=kB, vt=vt)

          def _attn(b, g):
            x_sb, qA, qB, kA, kB, vt = (S[b][k] for k in
                ("x_sb", "qA", "qB", "kA", "kB", "vt"))
            if g == 0:
                S[b]["zfA"] = zfp.tile([128, L], bf16, name="zfA")
                S[b]["zfB"] = zfp.tile([128, L], bf16, name="zfB")
            zfA, zfB = S[b]["zfA"], S[b]["zfB"]
            if True:
                q_t = (qA, qB)[g]
                k_t = (kA, kB)[g]
                zf = (zfA, zfB)[g]
                for j in range(2):
                    sj = slice(512 * j, 512 * j + 512)
                    if denmerge:
                        # comb[pp]: per head pair: [z_even | den_even | z_odd | den_odd]
                        combs = [pacc.tile([128, 512], f32, name="comb", bufs=cmbufs)
                                 for _ in range(2)]
                    else:
                        zden = pacc.tile([128, 512], f32, name="zden")
                        denb = pacc.tile([128, 512], f32, name="denb")
                    st8s = {}
                    for i in range(8):
                        p_i, ic = i >> 1, i & 1
                        sts = []
                        for pp in range(2):  # head pairs within group
                            r0 = 64 * pp
                            pl = pbig.tile([128, 2, 512], f32, name="pl")
                            for hh in range(2):
                                rr = r0 + 32 * hh
                                nc.tensor.matmul(
                                    out=pl[:, hh, :],
                                    lhsT=k_t[rr : rr + 32, 128 * i : 128 * i + 128],
                                    rhs=q_t[rr : rr + 32, sj],
                                    start=True, stop=True,
                                    tile_position=(rr, 0),
                                )
                            if (p_i, pp) in dve_pairs:
                                sti = stp.tile([128, 2, 512], i16, name="sti")
                                nc.vector.tensor_scalar(
                                    out=sti, in0=pl, scalar1=SCH_A,
                                    scalar2=SCH_B, op0=mult, op1=add,
                                )
                                sts.append(("bf", sti.bitcast(bf16)))
                            elif use_dr:
                                # exact exp (shifted) into the fp8 pair tile
                                if ic == 0:
                                    st8s[pp] = stp.tile([128, 2, 2, 512], f8,
                                                        name="st8")
                                st8 = st8s[pp]
                                nc.scalar.activation(out=st8[:, :, ic, :],
                                                     in_=pl, func=Exp,
                                                     bias=shift_sb[:, 0:1])
                                sts.append(("f8", st8))
                            else:
                                st = stp.tile([128, 2, 512], bf16, name="st")
                                nc.scalar.activation(out=st, in_=pl, func=Exp)
                                sts.append(("bf", st))
                        if denmerge:
                            for pp in range(2):
                                kind, st = sts[pp]
                                hp = 2 * g + pp
                                for hh in range(2):
                                    if kind == "f8":
                                        # full-width zero-padded lhsT: each DR
                                        # matmul writes all 128 comb rows; the
                                        # two heads accumulate
                                        if ic == 1:
                                            nc.tensor.matmul(
                                                out=combs[pp],
                                                lhsT=vt[:, p_i, :, hp, hh, :],
                                                rhs=st[:, hh, :, :],
                                                start=(p_i == 0 and hh == 0),
                                                stop=(p_i == 3 and hh == 1),
                                                skip_group_check=True,
                                                perf_mode=DR,
                                            )
                                    else:
                                        lhsT = (vt[:, p_i, ic, hp, hh,
                                                   64 * hh : 64 * hh + 64]
                                                if use_dr
                                                else vt[:, i, 4 * g + 2 * pp + hh, :])
                                        nc.tensor.matmul(
                                            out=combs[pp][64 * hh : 64 * hh + 64, :],
                                            lhsT=lhsT,
                                            rhs=st[:, hh, :],
                                            start=(i == 0), stop=(i == 7),
                                            tile_position=(0, 64 * hh),
                                            skip_group_check=True,
                                        )
                        else:
                            for hl in range(4):  # head-local index in group
                                _, st = sts[hl // 2]
                                mv = st[:, hl % 2, :]
                                nc.tensor.matmul(
                                    out=zden[32 * hl : 32 * hl + 32, :],
                                    lhsT=vt[:, i, 4 * g + hl, :],
                                    rhs=mv,
                                    start=(i == 0), stop=(i == 7),
                                    tile_position=(0, 32 * hl),
                                    skip_group_check=True,
                                )
                                nc.tensor.matmul(
                                    out=denb[32 * hl : 32 * hl + 32, :],
                                    lhsT=ones_sb,
                                    rhs=mv,
                                    start=(i == 0), stop=(i == 7),
                                    tile_position=(0, 32 * hl),
                                    skip_group_check=True,
                                )
                    if denmerge:
                        for pp in range(2):
                            rb = rbp.tile([128, 512], f32, name="rb")
                            nc.vector.reciprocal(out=rb, in_=combs[pp])
                            for hh in range(2):
                                h4 = (2 * pp + hh) % 4
                                nc.vector.tensor_tensor(
                                    out=zf[32 * h4 : 32 * h4 + 32, sj],
                                    in0=combs[pp][64 * hh : 64 * hh + 32, :],
                                    in1=rb[64 * hh + 32 : 64 * hh + 64, :],
                                    op=mult,
                                )
                    elif _stage_ge("norm"):
                        rb = rbp.tile([128, 512], f32, name="rb")
                        nc.vector.reciprocal(out=rb, in_=denb)
                        nc.vector.tensor_tensor(
                            out=zf[:, sj], in0=zden, in1=rb, op=mult
                        )
                    else:
                        nc.vector.tensor_copy(out=zf[:, sj], in_=zden)


          def _wo(b):
            x_sb, zfA, zfB = (S[b][k] for k in ("x_sb", "zfA", "zfB"))
            # ---- output projection + residual projection + bias
            po = pbig.tile([128, L], f32, name="pl")
            for j in range(2):
                sj = slice(512 * j, 512 * j + 512)
                nc.tensor.matmul(out=po[:, sj], lhsT=wo_sb[:, 0, :], rhs=zfA[:, sj],
                                 start=True, stop=False)
                nc.tensor.matmul(out=po[:, sj], lhsT=wo_sb[:, 1, :], rhs=zfB[:, sj],
                                 start=False, stop=False)
                nc.tensor.matmul(out=po[:, sj], lhsT=wo_sb[:, 2, :], rhs=x_sb[:, sj],
                                 start=False, stop=True)
            o_sb = outp.tile([128, L], f32, name="o_sb")
            if wo_act:
                nc.scalar.activation(out=o_sb, in_=po, func=Ident,
                                     bias=bout_sb[:, 0:1])
            else:
                nc.vector.tensor_scalar_add(out=o_sb, in0=po,
                                            scalar1=bout_sb[:, 0:1])
            nc.sync.dma_start(out=out_d[b], in_=o_sb)

          for b in range(BLOC):
            _proj(b)
          for b in range(BLOC):
            _attn(b, 0)
            _attn(b, 1)
          for b in range(BLOC):
            _wo(b)

    _split_excess_waits(nc, mybir)
    nc.finalize()
    return nc


def get_nc():
    if "nc" not in _CACHE:
        _CACHE["nc"] = _build_nc()
    return _CACHE["nc"]


def prep_weights(w_qkv, b_qkv, w_o, b_o, w_res, b_res):
    w_qkv = np.asarray(w_qkv, np.float32)
    b_qkv = np.asarray(b_qkv, np.float32)
    w_o = np.asarray(w_o, np.float32)
    b_o = np.asarray(b_o, np.float32)
    w_res = np.asarray(w_res, np.float32)
    b_res = np.asarray(b_res, np.float32)

    d = np.arange(32)
    qrows = np.concatenate([96 * h + d for h in range(H)])        # (256,)
    krows = np.concatenate([96 * h + 32 + d for h in range(H)])
    vrows = np.concatenate([96 * h + 64 + d for h in range(H)])

    Wq = w_qkv[qrows] * SCALE                                     # (256, C)
    Wk = w_qkv[krows]
    wqk = np.stack([Wq[:128].T, Wq[128:].T, Wk[:128].T, Wk[128:].T], axis=1)
    bqk = np.stack([b_qkv[qrows[:128]], b_qkv[qrows[128:]]], axis=1) * SCALE
    wv = np.ascontiguousarray(w_qkv[vrows].T)                     # (C, 256)
    wo = np.stack([w_o[:, :128].T, w_o[:, 128:].T, w_res.T], axis=1)
    bv = b_qkv[vrows]
    bout = (b_o + b_res + w_o @ bv)[:, None]

    import ml_dtypes
    bf = ml_dtypes.bfloat16
    return {
        "wqk": np.ascontiguousarray(wqk, bf),
        "bqk": np.ascontiguousarray(bqk, np.float32),
        "wv": np.ascontiguousarray(wv, bf),
        "wo": np.ascontiguousarray(wo, bf),
        "bout": np.ascontiguousarray(bout, np.float32),
    }


def make_in_maps(x, weights):
    import ml_dtypes
    x = np.ascontiguousarray(np.asarray(x).astype(ml_dtypes.bfloat16))
    return [
        dict(x_sh=np.ascontiguousarray(x[BLOC * i : BLOC * i + BLOC]), **weights)
        for i in range(NCORES)
    ]


class Runner:
    """Persistent PJRT executable for the SPMD bass program (axon path).

    Mirrors concourse.bass2jax.run_bass_via_pjrt's multi-core branch, but keeps
    the jitted callable so repeated executions don't re-trace/re-compile —
    needed both for a fast kernel() and for timing loops in test.py.
    """

    def __init__(self, nc=None, donate=True):
        import jax
        import concourse.mybir as mybir
        from concourse import bass2jax
        from jax.experimental.shard_map import shard_map
        from jax.sharding import Mesh, PartitionSpec

        if nc is None:
            nc = get_nc()
        bass2jax.install_neuronx_cc_hook()

        in_names, out_names, out_avals = [], [], []
        partition_name = (
            nc.partition_id_tensor.name if nc.partition_id_tensor else None
        )
        for alloc in nc.m.functions[0].allocations:
            if not isinstance(alloc, mybir.MemoryLocationSet):
                continue
            name = alloc.memorylocations[0].name
            if alloc.kind == "ExternalInput":
                if name != partition_name:
                    in_names.append(name)
            elif alloc.kind == "ExternalOutput":
                shape = tuple(alloc.tensor_shape)
                dtype = mybir.dt.np(alloc.dtype)
                out_avals.append(jax.core.ShapedArray(shape, dtype))
                out_names.append(name)
        n_params = len(in_names)
        n_outs = len(out_avals)
        all_in_names = list(in_names) + list(out_names)
        if partition_name is not None:
            all_in_names.append(partition_name)
        self.in_names = in_names
        self.out_names = out_names
        self.out_avals = out_avals

        donate_idx = tuple(range(n_params, n_params + n_outs)) if donate else ()

        def _body(*args):
            operands = list(args)
            if partition_name is not None:
                operands.append(bass2jax.partition_id_tensor())
            outs = bass2jax._bass_exec_p.bind(
                *operands,
                out_avals=tuple(out_avals),
                in_names=tuple(all_in_names),
                out_names=tuple(out_names),
                lowering_input_output_aliases=(),
                sim_require_finite=True,
                sim_require_nnan=True,
                nc=nc,
            )
            return tuple(outs)

        devices = jax.devices()[:NCORES]
        assert len(devices) == NCORES
        mesh = Mesh(np.asarray(devices), ("core",))
        in_specs = (PartitionSpec("core"),) * (n_params + n_outs)
        out_specs = (PartitionSpec("core"),) * n_outs
        self.sharded = jax.jit(
            shard_map(_body, mesh=mesh, in_specs=in_specs, out_specs=out_specs,
                      check_rep=False),
            donate_argnums=donate_idx,
            keep_unused=True,
        )
        self.mesh = mesh

    def prep(self, in_maps):
        return [
            np.concatenate([np.asarray(m[name]) for m in in_maps], axis=0)
            for name in self.in_names
        ]

    def zeros(self):
        return [
            np.zeros((NCORES * a.shape[0], *a.shape[1:]), a.dtype)
            for a in self.out_avals
        ]

    def call_async(self, concat_in):
        return self.sharded(*concat_in, *self.zeros())

    def __call__(self, in_maps):
        outs = self.call_async(self.prep(in_maps))
        arr = np.asarray(outs[0])
        return arr.reshape(NCORES, *self.out_avals[0].shape)


def get_runner():
    if "runner" not in _CACHE:
        _CACHE["runner"] = Runner()
    return _CACHE["runner"]


def run(x, weights, **kw):
    runner = get_runner()
    per_core = runner(make_in_maps(x, weights))
    out = per_core.reshape(B, C, L)
    return out, None


def kernel(x, w_qkv, b_qkv, w_o, b_o, w_res, b_res):
    weights = prep_weights(w_qkv, b_qkv, w_o, b_o, w_res, b_res)
    out, _ = run(x, weights)
    return out

